# revision 5
# baseline (speedup 1.0000x reference)
"""AggrGATConv Trainium2 kernel v3: int16 message stream + engine-split
identity aggregation.

Design (per-core, dst-sharded identity layout as v2):
  inv-1: h = feat @ W (fp32 2-pass exact) -> per-(node,head) int16
    quantization q = round(h * 32767 / max|h|), scale table sg, el/er tables.
  host: index-only slotting (deg-sorted octets, snake) + gathers:
    q16[src] rows into per-window blobs (layout per assigned reduce engine),
    el/sg[src] + er into a small resident aux stream.
  inv-2: per window w with T tiles:
    smalls (T-group batched): lg = el + er_bc; e = max(exp(lg), exp(.2 lg));
      s4 = sum_t e; r4 = 0.25/s4; es = e * sg * r4_bc  (alpha/4, sg-folded)
    big: wmsg = q * es_bc   (DVE or GpSimd per-window split)
         out chunk: PE identity-matmul 4-tile PSUM quadrants + XY-reduce
                    (PE windows)  OR  one fused XY-reduce on DVE.
    out: batched +bias, one output DMA.
"""
import sys
import types
import contextlib
import ctypes
import os

import numpy as np

import concourse.bacc as bacc
import concourse.tile as tile
import concourse.mybir as mybir
from concourse.bass_utils import run_bass_kernel_spmd

# ---------------- constants (hardcoded per problem spec) ----------------
N = 100000
E = 1600000
IN = 128
H, D = 4, 32
HD = H * D  # 128
NEG = 0.2
NCORES = 8
P = 128
K_WIN = 98                       # octets: 100352 / 1024
N_PAD = NCORES * K_WIN * P       # 100352
NODES_PER_CORE = K_WIN * P       # 12544
PAD_LOGIT = -100.0
FLOOR_LOGIT = -69.07755  # 5*ln(1e-6): keeps s4 >= 1e-6 on padded rows
QMAX = 32767.0

# engine-split ratios (tunable): per scheduled-window index
GPS_MULT_PAT, GPS_MULT_N = 12, 7   # gps mult if (i % PAT) < Nv
PE_RED_PAT, PE_RED_N = 5, 3        # pe reduce if (i % PAT) < Nv

f32 = mybir.dt.float32
i16 = mybir.dt.int16

Exp = mybir.ActivationFunctionType.Exp
Copy = mybir.ActivationFunctionType.Copy
Add = mybir.AluOpType.add
Mult = mybir.AluOpType.mult
Max = mybir.AluOpType.max
AbsMax = mybir.AluOpType.abs_max


def _install_ntff_shim():
    """antenv.axon_hooks is absent in this image; provide the ctypes hook so
    trace=True works (used by test harness; harmless otherwise)."""
    if "antenv.axon_hooks" in sys.modules:
        return
    try:
        lib = ctypes.CDLL("/opt/axon/libaxon_pjrt.so")
        if not hasattr(lib, "axon_start_nrt_profile"):
            raise OSError("no symbol")
        lib.axon_start_nrt_profile.argtypes = [
            ctypes.POINTER(ctypes.c_int64), ctypes.c_size_t]
        lib.axon_start_nrt_profile.restype = ctypes.c_int64
        lib.axon_stop_nrt_profile.argtypes = [ctypes.c_char_p]
        lib.axon_stop_nrt_profile.restype = ctypes.c_int64

        @contextlib.contextmanager
        def _hook(output_dir, device_ids):
            import jax
            jax.devices()
            if device_ids:
                ids = (ctypes.c_int64 * len(device_ids))(*device_ids)
                rc = lib.axon_start_nrt_profile(ids, len(device_ids))
            else:
                rc = lib.axon_start_nrt_profile(None, 0)
            if rc != 0:
                raise RuntimeError(f"axon_start_nrt_profile rc={rc}")
            try:
                yield
            finally:
                n = lib.axon_stop_nrt_profile(str(output_dir).encode())
                print(f"profile: {n} file(s) -> {output_dir}", file=sys.stderr)

        hook = _hook
    except OSError:
        hook = None
    mod = types.ModuleType("antenv.axon_hooks")
    mod.get_axon_ntff_profile_hook = lambda: hook
    mod.set_axon_ntff_profile_hook = lambda h: None
    sys.modules["antenv.axon_hooks"] = mod


_install_ntff_shim()


# ---------------- invocation 1: node tables + int16 quantization ---------
def _build_inv1():
    nc = bacc.Bacc("TRN2", target_bir_lowering=False, debug=False,
                   num_devices=NCORES)
    featT = nc.declare_dram_parameter("featT", [P, NODES_PER_CORE], f32,
                                      isOutput=False)
    W_in = nc.declare_dram_parameter("W", [IN, HD], f32, isOutput=False)
    WT_in = nc.declare_dram_parameter("WT", [HD, IN], f32, isOutput=False)
    Al_in = nc.declare_dram_parameter("Al", [HD, 4], f32, isOutput=False)
    Ar_in = nc.declare_dram_parameter("Ar", [HD, 4], f32, isOutput=False)
    # partition-major outputs; host reshapes back to node-major for free
    q_out = nc.declare_dram_parameter("q_out", [P, K_WIN * HD], i16,
                                      isOutput=True)
    elr_out = nc.declare_dram_parameter("elr_out", [P, K_WIN * 8], f32,
                                        isOutput=True)
    sg_out = nc.declare_dram_parameter("sg_out", [P, K_WIN * 4], f32,
                                       isOutput=True)

    with tile.TileContext(nc) as tc:
        with tc.tile_pool(name="cst", bufs=1) as cst, \
             tc.tile_pool(name="sb", bufs=3) as sb, \
             tc.tile_pool(name="ps", bufs=3, space="PSUM") as ps, \
             tc.tile_pool(name="psw", bufs=1, space="PSUM") as psw:

            # WLR = [W | Wl | Wr] where Wl = W @ Al, Wr = W @ Ar
            wt_sb = cst.tile([HD, IN], f32, tag="wt")
            nc.sync.dma_start(out=wt_sb[:], in_=WT_in[:])
            al_sb = cst.tile([HD, 4], f32, tag="al")
            nc.sync.dma_start(out=al_sb[:], in_=Al_in[:])
            ar_sb = cst.tile([HD, 4], f32, tag="ar")
            nc.sync.dma_start(out=ar_sb[:], in_=Ar_in[:])

            wlr = cst.tile([IN, 136], f32, tag="wlr")
            nc.sync.dma_start(out=wlr[:, 0:HD], in_=W_in[:])
            wl_ps = psw.tile([IN, 8], f32, tag="wlp")
            nc.tensor.matmul(out=wl_ps[:, 0:4], lhsT=wt_sb[:], rhs=al_sb[:],
                             start=True, stop=True)
            nc.tensor.matmul(out=wl_ps[:, 4:8], lhsT=wt_sb[:], rhs=ar_sb[:],
                             start=True, stop=True)
            nc.scalar.activation(out=wlr[:, 128:136], in_=wl_ps[:],
                                 func=Copy)

            CH = 14  # tiles per chunk; 98 = 7 chunks of 14
            n_chunks = NODES_PER_CORE // (P * CH)
            for c in range(n_chunks):
                ft = sb.tile([P, CH * P], f32, tag="ft")
                nc.sync.dma_start(
                    out=ft[:], in_=featT[:, c * CH * P:(c + 1) * CH * P])
                hsb = sb.tile([P, CH * 136], f32, tag="hsb")
                for t in range(CH):
                    hp = ps.tile([P, 136], f32, tag="hp")
                    # fp32 matmul mode is exact (2 half-speed passes)
                    nc.tensor.matmul(out=hp[:],
                                     lhsT=ft[:, t * P:(t + 1) * P],
                                     rhs=wlr[:], start=True, stop=True)
                    if t % 2 == 0:
                        nc.scalar.activation(
                            out=hsb[:, t * 136:(t + 1) * 136], in_=hp[:],
                            func=Copy)
                    else:
                        nc.vector.tensor_copy(hsb[:, t * 136:(t + 1) * 136],
                                              hp[:])
                # batched quantization over the chunk
                ga = hsb[:].rearrange("p (c f) -> p c f", c=CH)
                hview = ga[:, :, 0:128].rearrange(
                    "p c (hh d) -> p c hh d", hh=H)
                m4 = sb.tile([P, CH * 4], f32, tag="m4")
                nc.vector.tensor_reduce(
                    out=m4[:].rearrange("p (c h) -> p c h", c=CH),
                    in_=hview, axis=mybir.AxisListType.X, op=Max,
                    apply_absolute_value=True)
                sgc = sb.tile([P, CH * 4], f32, tag="sgc")
                nc.vector.tensor_scalar_mul(sgc[:], m4[:], 1.0 / QMAX)
                sinv = sb.tile([P, CH * 4], f32, tag="sinv")
                nc.vector.reciprocal(sinv[:], sgc[:])
                q16 = sb.tile([P, CH * HD], i16, tag="q16")
                nc.vector.tensor_tensor(
                    out=q16[:].rearrange("p (c hh d) -> p c hh d",
                                         c=CH, hh=H),
                    in0=hview,
                    in1=sinv[:].rearrange("p (c h) -> p c h", c=CH)
                        .unsqueeze(3).to_broadcast([P, CH, H, D]),
                    op=Mult)
                elrs = sb.tile([P, CH * 8], f32, tag="elrs")
                nc.gpsimd.tensor_copy(
                    elrs[:].rearrange("p (c e) -> p c e", c=CH),
                    ga[:, :, 128:136])
                nc.gpsimd.dma_start(
                    out=q_out[:, c * CH * HD:(c + 1) * CH * HD], in_=q16[:])
                nc.gpsimd.dma_start(
                    out=elr_out[:, c * CH * 8:(c + 1) * CH * 8], in_=elrs[:])
                nc.gpsimd.dma_start(
                    out=sg_out[:, c * CH * 4:(c + 1) * CH * 4], in_=sgc[:])
    nc.compile()
    return nc


def _schedule(Ts):
    """Schedule windows ASC by T; equal-T runs become batch groups.
    Returns (order, groups, meta) with stream offsets + engine classes."""
    Ts = list(Ts)
    order = [int(x) for x in np.argsort(np.asarray(Ts), kind="stable")]
    groups = []
    i = 0
    while i < len(order):
        j = i
        tval = Ts[order[i]]
        while j < len(order) and Ts[order[j]] == tval:
            j += 1
        groups.append((int(tval), order[i:j]))
        i = j
    q_off = {}
    off = 0
    for w in order:
        q_off[w] = off
        off += Ts[w] * HD
    aux_off = {}
    aoff = 0
    for w in order:
        aux_off[w] = aoff
        aoff += 8 * Ts[w] + 4
    use_pe = {}
    use_gps = {}
    for i, w in enumerate(order):
        use_pe[w] = (i % PE_RED_PAT) < PE_RED_N
        use_gps[w] = (i % GPS_MULT_PAT) < GPS_MULT_N
    meta = dict(order=order, groups=groups, q_off=q_off, aux_off=aux_off,
                CAPQ=off, CAPA=aoff, use_pe=use_pe, use_gps=use_gps)
    return meta


# ---------------- invocation 2: edge aggregation ----------------
def _build_inv2(Ts):
    """Ts = per-window tile counts in OCTET order (same on all cores)."""
    meta = _schedule(Ts)
    order = meta["order"]
    groups = meta["groups"]
    q_off = meta["q_off"]
    aux_off = meta["aux_off"]
    CAPQ, CAPA = meta["CAPQ"], meta["CAPA"]
    nsched = len(order)
    sched_pos = {w: i for i, w in enumerate(order)}

    nc = bacc.Bacc("TRN2", target_bir_lowering=False, debug=False,
                   num_devices=NCORES)
    q_d = nc.declare_dram_parameter("q", [P, CAPQ], i16, isOutput=False)
    aux_d = nc.declare_dram_parameter("aux", [P, CAPA], f32, isOutput=False)
    ident_d = nc.declare_dram_parameter("ident", [P, P], f32, isOutput=False)
    bias_in = nc.declare_dram_parameter("bias", [1, HD], f32, isOutput=False)
    out_d = nc.declare_dram_parameter("out", [P, K_WIN * D], f32,
                                      isOutput=True)

    with tile.TileContext(nc) as tc:
        with tc.tile_pool(name="cst", bufs=1) as cst, \
             tc.tile_pool(name="ld", bufs=3) as ld, \
             tc.tile_pool(name="wk", bufs=2) as wk, \
             tc.tile_pool(name="fl", bufs=2) as fl, \
             tc.tile_pool(name="ps", bufs=3, space="PSUM") as ps, \
             tc.tile_pool(name="psb", bufs=1, space="PSUM") as psb:

            ident = cst.tile([P, P], f32, tag="ident")
            nc.sync.dma_start(out=ident[:], in_=ident_d[:])

            # ---- constants: bias head-mean broadcast [P, D] ----
            bias_sb = cst.tile([1, HD], f32, tag="brow")
            nc.sync.dma_start(out=bias_sb[:], in_=bias_in[:])
            bias_m = cst.tile([1, D], f32, tag="bm")
            nc.vector.tensor_reduce(
                out=bias_m[:],
                in_=bias_sb[0:1, :].rearrange("p (h d) -> p d h", h=H),
                axis=mybir.AxisListType.X, op=Add)
            nc.vector.tensor_scalar_mul(bias_m[:], bias_m[:], 1.0 / H)
            ones1 = cst.tile([1, P], f32, tag="ones")
            nc.vector.memset(ones1[:], 1.0)
            bias_ps = psb.tile([P, D], f32, tag="bps")
            nc.tensor.matmul(out=bias_ps[:], lhsT=ones1[:], rhs=bias_m[:],
                             start=True, stop=True)
            bias_bc = cst.tile([P, D], f32, tag="bbc")
            nc.vector.tensor_copy(bias_bc[:], bias_ps[:])

            # resident aux stream (el/sg/er), ~52KB/partition
            aux = cst.tile([P, CAPA], f32, tag="aux")
            nchunk = 4
            csz = (CAPA + nchunk - 1) // nchunk
            for i in range(nchunk):
                lo = i * csz
                hi = min(CAPA, lo + csz)
                nc.sync.dma_start(out=aux[:, lo:hi], in_=aux_d[:, lo:hi])

            # output accumulator [P, nsched*D] (scheduled-window order)
            outbuf = cst.tile([P, nsched * D], f32, tag="outbuf")

            for (T, ws) in groups:
                nk = len(ws)
                a0 = aux_off[ws[0]]
                AW = 8 * T + 4
                ga = aux[:, a0:a0 + nk * AW].rearrange(
                    "p (k f) -> p k f", k=nk)
                el_v = ga[:, :, 0:4 * T].rearrange(
                    "p k (h t) -> p k h t", h=H)
                sg_v = ga[:, :, 4 * T:8 * T].rearrange(
                    "p k (h t) -> p k h t", h=H)
                er_v = ga[:, :, 8 * T:8 * T + 4]

                # ---- batched smalls over the group ----
                lg = fl.tile([P, nk * 4 * T], f32, tag="lg")
                nc.vector.tensor_tensor(
                    out=lg[:].rearrange("p (k h t) -> p k h t", k=nk, h=H),
                    in0=el_v,
                    in1=er_v.unsqueeze(3).to_broadcast([P, nk, H, T]),
                    op=Add)
                e1 = fl.tile([P, nk * 4 * T], f32, tag="e1")
                nc.scalar.activation(out=e1[:], in_=lg[:], func=Exp)
                e2 = fl.tile([P, nk * 4 * T], f32, tag="e2")
                nc.scalar.activation(out=e2[:], in_=lg[:], scale=NEG,
                                     func=Exp)
                ee = fl.tile([P, nk * 4 * T], f32, tag="ee")
                nc.vector.tensor_tensor(out=ee[:], in0=e1[:], in1=e2[:],
                                        op=Max)
                s4 = fl.tile([P, nk * 4], f32, tag="s4")
                nc.vector.tensor_reduce(
                    out=s4[:],
                    in_=ee[:].rearrange("p (f t) -> p f t", t=T),
                    axis=mybir.AxisListType.X, op=Add)
                r4 = fl.tile([P, nk * 4], f32, tag="r4")
                nc.vector.reciprocal(r4[:], s4[:])
                nc.vector.tensor_scalar_mul(r4[:], r4[:], 1.0 / H)
                sgr = fl.tile([P, nk * 4 * T], f32, tag="sgr")
                nc.vector.tensor_tensor(
                    out=sgr[:].rearrange("p (k h t) -> p k h t", k=nk, h=H),
                    in0=sg_v,
                    in1=r4[:].rearrange("p (k h) -> p k h", k=nk)
                        .unsqueeze(3).to_broadcast([P, nk, H, T]),
                    op=Mult)
                es = fl.tile([P, nk * 4 * T], f32, tag="es")
                nc.vector.tensor_tensor(out=es[:], in0=ee[:], in1=sgr[:],
                                        op=Mult)

                # ---- per-window heavy ops ----
                for ki, w in enumerate(ws):
                    use_gps = meta["use_gps"][w]
                    use_pe = meta["use_pe"][w]
                    KW = T * HD
                    qt = ld.tile([P, KW], i16, tag="qt")
                    nc.sync.dma_start(out=qt[:],
                                      in_=q_d[:, q_off[w]:q_off[w] + KW])
                    esw = es[:, ki * 4 * T:(ki + 1) * 4 * T]
                    wmsg = wk.tile([P, KW], f32, tag="wmsg")
                    meng = nc.gpsimd if use_gps else nc.vector
                    sp = sched_pos[w]
                    if use_pe:
                        # q layout (t, h, d); es stays (h, t)-major
                        meng.tensor_tensor(
                            out=wmsg[:].rearrange("p (t hh d) -> p t hh d",
                                                  t=T, hh=H),
                            in0=qt[:].rearrange("p (t hh d) -> p t hh d",
                                                t=T, hh=H),
                            in1=esw.rearrange("p (h t) -> p t h", h=H)
                                .unsqueeze(3).to_broadcast([P, T, H, D]),
                            op=Mult)
                        n4 = T // 4
                        tail = T % 4
                        u4 = ps.tile([P, 512], f32, tag="u4")
                        for g in range(n4):
                            nc.tensor.matmul(
                                out=u4[:], lhsT=ident[:],
                                rhs=wmsg[:, g * 512:(g + 1) * 512],
                                start=(g == 0),
                                stop=(g == n4 - 1 and tail == 0))
                        for x in range(tail):
                            nc.tensor.matmul(
                                out=u4[:, 0:HD], lhsT=ident[:],
                                rhs=wmsg[:, (n4 * 4 + x) * HD:
                                         (n4 * 4 + x + 1) * HD],
                                start=(n4 == 0 and x == 0),
                                stop=(x == tail - 1))
                        # out[p, d] = sum over (quadrant, head)
                        nc.vector.tensor_reduce(
                            out=outbuf[:, sp * D:(sp + 1) * D],
                            in_=u4[:].rearrange("p (q hh d) -> p d q hh",
                                                q=4, hh=H),
                            axis=mybir.AxisListType.XY, op=Add)
                    else:
                        # q layout (h, d, t); es (h, t)-major
                        meng.tensor_tensor(
                            out=wmsg[:].rearrange("p (hh d t) -> p hh d t",
                                                  hh=H, d=D),
                            in0=qt[:].rearrange("p (hh d t) -> p hh d t",
                                                hh=H, d=D),
                            in1=esw.rearrange("p (h t) -> p h t", h=H)
                                .unsqueeze(2).to_broadcast([P, H, D, T]),
                            op=Mult)
                        nc.vector.tensor_reduce(
                            out=outbuf[:, sp * D:(sp + 1) * D],
                            in_=wmsg[:].rearrange("p (hh d t) -> p d hh t",
                                                  hh=H, d=D),
                            axis=mybir.AxisListType.XY, op=Add)

            # ---- finalize: += bias (in place), one store ----
            nc.vector.tensor_tensor(
                out=outbuf[:].rearrange("p (k d) -> p k d", k=nsched),
                in0=outbuf[:].rearrange("p (k d) -> p k d", k=nsched),
                in1=bias_bc[:].unsqueeze(1).to_broadcast([P, nsched, D]),
                op=Add)
            nc.gpsimd.dma_start(out=out_d[:, 0:nsched * D], in_=outbuf[:])
    nc.compile()
    return nc, meta


_INV1 = None
_INV2 = {}
LAST_EXEC_NS = None
LAST_EXEC_NS1 = None
LAST_EXEC_NS2 = None
_TRACE = bool(os.environ.get("GAT_TRACE"))


def kernel(feat, W, attn_l, attn_r, bias, src, dst):
    global _INV1, LAST_EXEC_NS, LAST_EXEC_NS1, LAST_EXEC_NS2
    feat = np.asarray(feat, dtype=np.float32)
    W = np.asarray(W, dtype=np.float32)
    attn_l = np.asarray(attn_l, dtype=np.float32)
    attn_r = np.asarray(attn_r, dtype=np.float32)
    bias = np.asarray(bias, dtype=np.float32)
    src = np.asarray(src, dtype=np.int32)
    dst = np.asarray(dst, dtype=np.int32)

    # ---------------- host: layout-only prep ----------------
    featT = np.zeros((IN, N_PAD), dtype=np.float32)
    featT[:, :N] = np.ascontiguousarray(feat.T)
    WT = np.ascontiguousarray(W.T)
    Al = np.zeros((HD, H), dtype=np.float32)
    Ar = np.zeros((HD, H), dtype=np.float32)
    for h in range(H):
        Al[h * D:(h + 1) * D, h] = attn_l[h]
        Ar[h * D:(h + 1) * D, h] = attn_r[h]

    # ---------------- inv-1 ----------------
    if _INV1 is None:
        _INV1 = _build_inv1()
    in1 = []
    for c in range(NCORES):
        sl = slice(c * NODES_PER_CORE, (c + 1) * NODES_PER_CORE)
        in1.append({"featT": np.ascontiguousarray(featT[:, sl]),
                    "W": W, "WT": WT, "Al": Al, "Ar": Ar})
    res1 = run_bass_kernel_spmd(_INV1, in1, core_ids=list(range(NCORES)),
                                trace=_TRACE)
    LAST_EXEC_NS1 = res1.exec_time_ns
    q_full = np.concatenate(
        [r["q_out"].reshape(P, K_WIN, HD).transpose(1, 0, 2)
         .reshape(NODES_PER_CORE, HD) for r in res1.results], axis=0)
    elr_full = np.concatenate(
        [r["elr_out"].reshape(P, K_WIN, 8).transpose(1, 0, 2)
         .reshape(NODES_PER_CORE, 8) for r in res1.results], axis=0)
    sg_full = np.concatenate(
        [r["sg_out"].reshape(P, K_WIN, 4).transpose(1, 0, 2)
         .reshape(NODES_PER_CORE, 4) for r in res1.results], axis=0)

    # ---------------- host: identity-layout slotting (index ops only) ----
    deg = np.bincount(dst, minlength=N_PAD).astype(np.int64)
    order_n = np.argsort(-deg, kind="stable")
    rank = np.empty(N_PAD, dtype=np.int64)
    rank[order_n] = np.arange(N_PAD)
    k_of = rank >> 10
    within = rank & 1023
    c_of = within >> 7
    c_of = np.where(k_of & 1 == 1, NCORES - 1 - c_of, c_of)  # snake
    p_of = within & 127

    Ts = deg[order_n[::1024]]
    Ts = np.maximum(Ts, 4)  # floor so every window has >= one 4-tile matmul
    key = tuple(int(t) for t in Ts)
    if key not in _INV2:
        _INV2[key] = _build_inv2(key)
    nc2, meta = _INV2[key]

    Ts_np = np.asarray(key, dtype=np.int64)
    nsched = len(meta["order"])
    sched_pos = np.empty(K_WIN, dtype=np.int64)
    sched_pos[np.asarray(meta["order"])] = np.arange(nsched)
    q_off = np.zeros(K_WIN, dtype=np.int64)
    aux_off = np.zeros(K_WIN, dtype=np.int64)
    for w in range(K_WIN):
        q_off[w] = meta["q_off"][w]
        aux_off[w] = meta["aux_off"][w]
    use_pe_w = np.zeros(K_WIN, dtype=bool)
    for w in range(K_WIN):
        use_pe_w[w] = meta["use_pe"][w]
    CAPQ, CAPA = meta["CAPQ"], meta["CAPA"]

    # per-edge slot computation
    perm = np.argsort(dst, kind="stable")
    dstp = dst[perm]
    srcp = src[perm]
    estart = np.zeros(N_PAD + 1, dtype=np.int64)
    np.cumsum(np.bincount(dstp, minlength=N_PAD), out=estart[1:])
    t_of = np.arange(E, dtype=np.int64) - estart[dstp]
    ce = c_of[dstp]
    pe_row = p_of[dstp]
    we = k_of[dstp]
    te = t_of
    Te = Ts_np[we]

    # ---- q stream: scatter q rows into per-window blobs ----
    q_lay = np.zeros((NCORES, P, CAPQ), dtype=np.int16)
    qflat = q_lay.reshape(-1)
    rowbase = (ce * P + pe_row) * CAPQ
    hdidx = np.arange(HD, dtype=np.int64)
    is_pe_e = use_pe_w[we]
    idx_pe = np.nonzero(is_pe_e)[0]
    idx_dv = np.nonzero(~is_pe_e)[0]
    # PE windows: (t, h, d) layout -> cols q_off + t*128 + hd
    cols = (rowbase[idx_pe] + q_off[we[idx_pe]]
            + te[idx_pe] * HD)[:, None] + hdidx[None, :]
    qflat[cols] = q_full[srcp[idx_pe]]
    del cols
    # DVE windows: (h, d, t) layout -> cols q_off + hd*T + t
    cols = (rowbase[idx_dv] + q_off[we[idx_dv]] + te[idx_dv])[:, None] \
        + hdidx[None, :] * Te[idx_dv][:, None]
    qflat[cols] = q_full[srcp[idx_dv]]
    del cols

    # ---- aux stream: el/sg (h,t)-major + er per window ----
    aux_lay = np.zeros((NCORES, P, CAPA), dtype=np.float32)
    # pad logits: default el region to PAD_LOGIT
    for w in range(K_WIN):
        a0 = aux_off[w]
        aux_lay[:, :, a0:a0 + 4 * Ts_np[w]] = PAD_LOGIT
    aflat = aux_lay.reshape(-1)
    rowbase_a = (ce * P + pe_row) * CAPA
    hidx = np.arange(H, dtype=np.int64)
    el_cols = (rowbase_a + aux_off[we] + te)[:, None] \
        + hidx[None, :] * Te[:, None]
    aflat[el_cols] = elr_full[srcp][:, 0:4]
    sg_cols = el_cols + 4 * Te[:, None]
    aflat[sg_cols] = sg_full[srcp]
    del el_cols, sg_cols
    # s-floor: one slot per row at t=deg (first padding slot)
    nodes = np.arange(N_PAD)
    has_pad = deg < Ts_np[k_of]
    rb_n = (c_of * P + p_of) * CAPA
    fl_cols = (rb_n + aux_off[k_of] + deg)[:, None] \
        + hidx[None, :] * Ts_np[k_of][:, None]
    aflat[fl_cols[has_pad]] = FLOOR_LOGIT
    # er per window row
    er_cols = (rb_n + aux_off[k_of] + 8 * Ts_np[k_of])[:, None] \
        + hidx[None, :]
    aflat[er_cols] = elr_full[nodes][:, 4:8]
    del fl_cols, er_cols

    ident = np.eye(P, dtype=np.float32)

    # ---------------- inv-2 ----------------
    in2 = []
    for c in range(NCORES):
        in2.append({"q": q_lay[c], "aux": aux_lay[c], "ident": ident,
                    "bias": bias.reshape(1, HD)})
    res2 = run_bass_kernel_spmd(nc2, in2, core_ids=list(range(NCORES)),
                                trace=_TRACE)
    LAST_EXEC_NS2 = res2.exec_time_ns
    if LAST_EXEC_NS1 is not None and LAST_EXEC_NS2 is not None:
        LAST_EXEC_NS = LAST_EXEC_NS1 + LAST_EXEC_NS2
    out_full = np.zeros((N_PAD, D), dtype=np.float32)
    res_arr = np.stack([r["out"].reshape(P, K_WIN, D)
                        for r in res2.results])  # [c, p, schedpos, d]
    out_full[nodes] = res_arr[c_of, p_of, sched_pos[k_of], :]
    return np.ascontiguousarray(out_full[:N])
